# revision 7
# baseline (speedup 1.0000x reference)
"""Channel-attention block kernel for Trainium2 (8 NeuronCores, SPMD).

Reference (per batch b):
    q = x[b]                       # [C=512, N=4096]  (N = H*W)
    aff = q @ q.T                  # [512, 512]
    attn = softmax(rowmax(aff) - aff, axis=-1)
         = exp(rowmin(aff) - aff) / rowsum(...)
    out[b] = gamma * (attn @ q) + x[b]

Strategy: data-parallel over B (16 batches / 8 cores = 2 per core).
Matmuls in float32r (TF32-like: full PE rate at N=512, ~1e-4 rel err).
Per batch on-core:
  1. gpsimd cast-DMA x -> SBUF q tiles [128, 4096] f32r (4 chunks of C)
  2. PE-transpose q -> qT k-chunks [128n, 512c] (transient)
  3. MM1: aff row-blocks [128, 512] in 4 PSUM banks, accum over 32 k-chunks
  4. DVE row-min, ACT exp(min - aff) with accumulated row-sum, reciprocal
  5. PE-transpose attn -> attnT; MM2 out-blocks [128c, 512n];
     epilogue out = (gamma*rz[c]) * psum + q  (ACT scale + DVE add), DMA out
"""

import numpy as np

import concourse.bacc as bacc
import concourse.bass as bass
import concourse.tile as tile
from concourse import mybir
from concourse.bass_utils import run_bass_kernel_spmd
from concourse.masks import make_identity

B, C, H, W = 16, 512, 64, 64
N = H * W            # 4096
NCORES = 8
BPC = B // NCORES    # batches per core
CP = C // 128        # 4 channel blocks
KP = N // 128        # 32 n-chunks
NJ = N // 512        # 8 output col blocks

f32 = mybir.dt.float32
f32r = mybir.dt.float32r


def _build_body(nc, tc, x, gamma, y):
    ctx_pools = {}

    def pool(name, bufs, space="SBUF"):
        ctx_pools[name] = tc.alloc_tile_pool(name=name, bufs=bufs, space=space)
        return ctx_pools[name]

    qr_p = pool("qr", BPC)                # [128, N] f32r, tag per i, BPC slots
    qt_p = pool("qt", 3)                  # [128, C] f32r transient transposes
    attn_p = pool("attn", 5)              # [128, C] f32r
    attnT_p = pool("attnT", 5)            # [128, C] f32r
    outsb_p = pool("outsb", 4)            # [128, 512] f32
    small_p = pool("small", 4 * 4)        # [128, 1] f32
    const_p = pool("const", 1)
    ps_t = pool("ps_t", 2, space="PSUM")    # transpose banks
    ps_aff = pool("ps_aff", 1, space="PSUM")  # tag per i -> 4 banks
    ps_out = pool("ps_out", 2, space="PSUM")

    # constants: fp32r identity for PE transposes, gamma broadcast
    ident_f = const_p.tile([128, 128], f32)
    make_identity(nc, ident_f)
    ident = const_p.tile([128, 128], f32r)
    nc.gpsimd.dma_start(out=ident, in_=ident_f)     # cast fp32 -> fp32r
    gamma_sb = const_p.tile([128, 1], f32)
    nc.gpsimd.dma_start(out=gamma_sb, in_=gamma.to_broadcast([128, 1]))

    for b in range(BPC):
        # ---- load q (rounded to f32r) ----
        q = []
        for i in range(CP):
            qi = qr_p.tile([128, N], f32r, tag=f"qr{i}")
            nc.gpsimd.dma_start(out=qi, in_=x[b, 128 * i:128 * (i + 1), :])
            q.append(qi)

        # ---- MM1: affinity, interleaved with qT production ----
        aff = [ps_aff.tile([128, C], f32, tag=f"aff{i}", name=f"aff{i}")
               for i in range(CP)]
        for k in range(KP):
            pst = ps_t.tile([128, C], f32r)
            for i in range(CP):
                nc.tensor.transpose(
                    pst[:, 128 * i:128 * (i + 1)],
                    q[i][:, 128 * k:128 * (k + 1)],
                    ident,
                )
            qt = qt_p.tile([128, C], f32r)
            nc.vector.tensor_copy(out=qt, in_=pst)
            for i in range(CP):
                nc.tensor.matmul(
                    aff[i],
                    qt[:, 128 * i:128 * (i + 1)],
                    qt,
                    start=(k == 0),
                    stop=(k == KP - 1),
                )

        # ---- softmax(min-centered, negated) ----
        attn = []
        grz = []
        for i in range(CP):
            m = small_p.tile([128, 1], f32, tag="m")
            nc.vector.tensor_reduce(
                out=m, in_=aff[i], op=mybir.AluOpType.min, axis=mybir.AxisListType.X
            )
            a_t = attn_p.tile([128, C], f32r)
            z = small_p.tile([128, 1], f32, tag="z")
            nc.scalar.activation(
                out=a_t, in_=aff[i], func=mybir.ActivationFunctionType.Exp,
                bias=m, scale=-1.0, accum_out=z,
            )
            rz = small_p.tile([128, 1], f32, tag="rz")
            nc.vector.reciprocal(out=rz, in_=z)
            g = small_p.tile([128, 1], f32, tag="grz")
            nc.vector.tensor_scalar_mul(out=g, in0=rz, scalar1=gamma_sb)
            attn.append(a_t)
            grz.append(g)

        # ---- attnT via PE transpose ----
        attnT = []
        for kd in range(CP):
            pst = ps_t.tile([128, C], f32r)
            for i in range(CP):
                nc.tensor.transpose(
                    pst[:, 128 * i:128 * (i + 1)],
                    attn[i][:, 128 * kd:128 * (kd + 1)],
                    ident,
                )
            at = attnT_p.tile([128, C], f32r)
            nc.vector.tensor_copy(out=at, in_=pst)
            attnT.append(at)

        # ---- MM2 + epilogue ----
        for i in range(CP):
            for j in range(NJ):
                po = ps_out.tile([128, 512], f32)
                for kd in range(CP):
                    nc.tensor.matmul(
                        po,
                        attnT[kd][:, 128 * i:128 * (i + 1)],
                        q[kd][:, 512 * j:512 * (j + 1)],
                        start=(kd == 0),
                        stop=(kd == CP - 1),
                    )
                o = outsb_p.tile([128, 512], f32)
                # o = (gamma / z[c]) * att_out
                nc.scalar.activation(
                    out=o, in_=po, func=mybir.ActivationFunctionType.Copy,
                    scale=grz[i],
                )
                # o += x (f32r-rounded residual; bit-identical layout)
                nc.vector.tensor_add(
                    out=o, in0=o,
                    in1=q[i][:, 512 * j:512 * (j + 1)].bitcast(f32),
                )
                nc.sync.dma_start(
                    out=y[b, 128 * i:128 * (i + 1), 512 * j:512 * (j + 1)],
                    in_=o,
                )

    for p in reversed(list(ctx_pools.values())):
        p.release()


_NC_CACHE = {}


def build_kernel(bpc=BPC):
    if bpc in _NC_CACHE:
        return _NC_CACHE[bpc]
    global BPC
    old_bpc, BPC = BPC, bpc
    try:
        nc = bacc.Bacc("TRN2", target_bir_lowering=False, debug=False, num_devices=1)
        x = nc.dram_tensor("x", [bpc, C, N], f32, kind="ExternalInput").ap()
        gamma = nc.dram_tensor("gamma", [1], f32, kind="ExternalInput").ap()
        y = nc.dram_tensor("y", [bpc, C, N], f32, kind="ExternalOutput").ap()
        with tile.TileContext(nc) as tc:
            _build_body(nc, tc, x, gamma, y)
        nc.compile()
    finally:
        BPC = old_bpc
    _NC_CACHE[bpc] = nc
    return nc


def run(x, gamma, trace=False):
    """x: [B, C, H, W] f32, gamma: [1] f32 -> ([B, C, H, W] f32, results)"""
    x = np.ascontiguousarray(x, dtype=np.float32).reshape(B, C, N)
    gamma = np.ascontiguousarray(gamma, dtype=np.float32)
    nc = build_kernel()
    in_maps = [
        {"x": x[i * BPC:(i + 1) * BPC], "gamma": gamma} for i in range(NCORES)
    ]
    res = run_bass_kernel_spmd(nc, in_maps, core_ids=list(range(NCORES)),
                               trace=trace)
    out = np.concatenate([res.results[i]["y"] for i in range(NCORES)], axis=0)
    return out.reshape(B, C, H, W), res


def kernel(x, gamma):
    out, _ = run(x, gamma)
    return out


# revision 37
# speedup vs baseline: 42.4293x; 42.4293x over previous
"""Channel-attention block kernel for Trainium2 (8 NeuronCores, SPMD).

Reference (per batch b):
    q = x[b]                       # [C=512, N=4096]  (N = H*W)
    aff = q @ q.T                  # [512, 512]
    attn = softmax(rowmax(aff) - aff, axis=-1)
         = exp(rowmin(aff) - aff) / rowsum(...)
    out[b] = gamma * (attn @ q) + x[b]

Strategy: data-parallel over B (16 batches / 8 cores = 2 per core).
Matmuls in float32r (TF32-like: full PE rate at N>=256, ~1e-4 rel err).
Per batch on-core:
  1. gpsimd cast-DMA x -> SBUF q quarter-tiles [128, 1024] f32r (4c x 4q)
  2. PE-transpose q -> qT k-chunks [128n, 512c] (transient), software-
     pipelined with lookahead 2; PSUM->SBUF evacuation alternates DVE/ACT
  3. MM1: aff row-blocks in 4 PSUM banks, accum over 32 k-chunks; only the
     upper block-triangle is computed (aff is symmetric), 3 lower blocks
     reconstructed by PE transpose
  4. DVE row-min, ACT exp(min - aff) with accumulated row-sum Z,
     attn *= gamma/Z (pre-scaled so the MM2 epilogue is just +x)
  5. PE-transpose attn -> attnT; MM2 out-blocks [128c, 512n];
     epilogue out = psum + q (single DVE add from PSUM), DMA out
Emission order: phase1(b0), phase1(b1), phase2(b0), phase2(b1) so each
softmax tail hides under the other batch's matmul stream.
"""

import numpy as np

import concourse.bacc as bacc
import concourse.tile as tile
from concourse import mybir
from concourse.bass_utils import run_bass_kernel_spmd
from concourse.masks import make_identity

B, C, H, W = 16, 512, 64, 64
N = H * W            # 4096
NQ = N // 4          # 1024 quarter-tile width
NCORES = 8
BPC = B // NCORES    # batches per core
CP = C // 128        # 4 channel blocks
KP = N // 128        # 32 n-chunks
NJ = N // 512        # 8 output col blocks

f32 = mybir.dt.float32
f32r = mybir.dt.float32r


def _build_body(nc, tc, x, gamma, y):
    pools = {}

    def pool(name, bufs, space="SBUF"):
        pools[name] = tc.alloc_tile_pool(name=name, bufs=bufs, space=space)
        return pools[name]

    qr_p = pool("qr", BPC)                # [128, NQ] f32r, tag per (i,h)
    qt_p = pool("qt", 4)                  # [128, C] f32r transient transposes
    attn_p = pool("attn", 5)              # [128, C] f32r
    attnT_p = pool("attnT", 8)            # [128, C] f32r
    outsb_p = pool("outsb", 6)            # [128, 512] f32
    small_p = pool("small", 8)            # [128, 1] f32
    const_p = pool("const", 1)
    ps_t = pool("ps_t", 2, space="PSUM")      # transpose evacuation banks
    ps_aff = pool("ps_aff", 1, space="PSUM")  # tag per i -> 4 banks
    ps_out = pool("ps_out", 2, space="PSUM")

    def load_q(b):
        # load q quarters (rounded to f32r); low quarters first
        q = {}
        for h in range(4):
            for i in range(CP):
                qi = qr_p.tile([128, NQ], f32r, tag=f"qr{i}{h}", name=f"qr{i}{h}")
                nc.gpsimd.dma_start(
                    out=qi,
                    in_=x[b, 128 * i:128 * (i + 1), NQ * h:NQ * (h + 1)],
                )
                q[(i, h)] = qi
        return q

    # HAM warmup: PE transposes of a memset scratch keep the PE busy while
    # the first loads land, so the clock gate opens before real matmuls.
    warm_in = const_p.tile([128, 128], f32)
    nc.vector.memset(warm_in, 1.0)
    warm_ps = ps_t.tile([128, C], f32, tag="pst", name="warm_ps")
    for w in range(16):
        nc.tensor.transpose(
            warm_ps[:, 128 * (w % CP):128 * (w % CP + 1)], warm_in, warm_in)
    warm_sb = const_p.tile([128, C], f32)
    nc.vector.tensor_copy(out=warm_sb, in_=warm_ps)

    # ident (first transpose needs it), then batch-0 loads, then gamma
    ident_f = const_p.tile([128, 128], f32)
    make_identity(nc, ident_f)
    ident = const_p.tile([128, 128], f32r)
    nc.gpsimd.dma_start(out=ident, in_=ident_f)     # cast fp32 -> fp32r
    q0 = load_q(0)
    gamma_sb = const_p.tile([128, 1], f32)
    nc.gpsimd.dma_start(out=gamma_sb, in_=gamma.to_broadcast([128, 1]))

    def phase1_gen(b, q, out_attnT):
        # MM1 software-pipelined (lookahead 2): transposes+copy for chunk
        # k+2 emitted before chunk k's matmuls so copy latency hides.
        qts = {}

        def emit_stage(k):
            h, kh = divmod(k, KP // 4)
            pst = ps_t.tile([128, C], f32r, tag="pst", name="pst")
            for i in range(CP):
                nc.tensor.transpose(
                    pst[:, 128 * i:128 * (i + 1)],
                    q[(i, h)][:, 128 * kh:128 * (kh + 1)],
                    ident,
                )
            qt = qt_p.tile([128, C], f32r, tag="qt", name="qt")
            if k % 2 == 0:
                nc.vector.tensor_copy(out=qt, in_=pst)
            else:
                nc.scalar.copy(out=qt, in_=pst)
            qts[k] = qt

        emit_stage(0)
        emit_stage(1)
        aff = [ps_aff.tile([128, C], f32, tag=f"aff{i}", name=f"aff{i}")
               for i in range(CP)]
        # affinity is symmetric: rows 0-2 compute cols >= 128*i only
        # (N=512/384/256); row 3 full (N=128 would hit the fp32r small-N
        # penalty). Missing lower blocks (1,0),(2,0),(2,1) reconstructed
        # from their transposes after the accumulation finishes.
        for k in range(KP):
            if k + 2 < KP:
                emit_stage(k + 2)
            qt = qts.pop(k)
            for i in range(CP):
                lo = 128 * i if i < CP - 1 else 0
                nc.tensor.matmul(
                    aff[i][:, lo:],
                    qt[:, 128 * i:128 * (i + 1)],
                    qt[:, lo:],
                    start=(k == 0),
                    stop=(k == KP - 1),
                )
            yield
        for (bi, bj) in [(1, 0), (2, 0), (2, 1)]:
            # aff[bi][:, bj-block] = aff[bj][:, bi-block].T
            tmp = qt_p.tile([128, 128], f32, tag="tri", name="tri")
            nc.scalar.copy(
                out=tmp, in_=aff[bj][:, 128 * bi:128 * (bi + 1)])
            nc.tensor.matmul(
                aff[bi][:, 128 * bj:128 * (bj + 1)],
                tmp, ident_f, is_transpose=True, skip_group_check=True,
            )

        # softmax(min-centered, negated), pre-scaled by gamma/Z
        # order 0,3 first: rows 0/3 need no triangle reconstruction
        attn = [None] * CP
        for i in (0, 3, 1, 2):
            m = small_p.tile([128, 1], f32, tag="m")
            nc.vector.tensor_reduce(
                out=m, in_=aff[i], op=mybir.AluOpType.min, axis=mybir.AxisListType.X
            )
            a_t = attn_p.tile([128, C], f32r, tag="a_t", name="a_t")
            z = small_p.tile([128, 1], f32, tag="z")
            nc.scalar.activation(
                out=a_t, in_=aff[i], func=mybir.ActivationFunctionType.Exp,
                bias=m, scale=-1.0, accum_out=z,
            )
            rz = small_p.tile([128, 1], f32, tag="rz")
            nc.vector.reciprocal(out=rz, in_=z)
            g = small_p.tile([128, 1], f32, tag="grz", name="grz")
            nc.vector.tensor_scalar_mul(out=g, in0=rz, scalar1=gamma_sb)
            nc.vector.tensor_scalar_mul(out=a_t, in0=a_t, scalar1=g)
            attn[i] = a_t
            yield

        # attnT via PE transpose; staging pool alternates per batch so
        # MM2(b0) psum allocs aren't FIFO-blocked behind attnT(b1)
        attnT = []
        stage_pool, stage_tag = (ps_out, "po") if b % 2 == 0 else (ps_t, "pst")
        for kd in range(CP):
            pst = stage_pool.tile([128, C], f32r, tag=stage_tag, name=stage_tag)
            for i in range(CP):
                nc.tensor.transpose(
                    pst[:, 128 * i:128 * (i + 1)],
                    attn[i][:, 128 * kd:128 * (kd + 1)],
                    ident,
                )
            at = attnT_p.tile([128, C], f32r, tag="at", name="at")
            nc.vector.tensor_copy(out=at, in_=pst)
            attnT.append(at)
            yield
        out_attnT.extend(attnT)

    def phase2_gen(b, q, attnT):
        # MM2 + epilogue (single DVE add from PSUM)
        last = (b == BPC - 1)
        for i in range(CP):
            for j in range(NJ):
                h, jh = divmod(j, NJ // 4)
                if last and (i * NJ + j) % 2 == 1:
                    po = ps_t.tile([128, 512], f32, tag="pst", name="pst")
                else:
                    po = ps_out.tile([128, 512], f32, tag="po", name="po")
                for kd in range(CP):
                    nc.tensor.matmul(
                        po,
                        attnT[kd][:, 128 * i:128 * (i + 1)],
                        q[(kd, h)][:, 512 * jh:512 * (jh + 1)],
                        start=(kd == 0),
                        stop=(kd == CP - 1),
                    )
                o = outsb_p.tile([128, 512], f32)
                nc.vector.tensor_add(
                    out=o, in0=po,
                    in1=q[(i, h)][:, 512 * jh:512 * (jh + 1)].bitcast(f32),
                )
                nc.sync.dma_start(
                    out=y[b, 128 * i:128 * (i + 1), 512 * j:512 * (j + 1)],
                    in_=o,
                )
                yield

    def drain(g):
        for _ in g:
            pass

    qs = {0: q0}
    attnTs = {}
    for b in range(BPC):
        if b > 0:
            qs[b] = load_q(b)
        attnTs[b] = []
        drain(phase1_gen(b, qs[b], attnTs[b]))
    for b in range(BPC):
        drain(phase2_gen(b, qs[b], attnTs[b]))

    for p in reversed(list(pools.values())):
        p.release()


_NC_CACHE = {}


def build_kernel(bpc=BPC, repeat=1):
    key = (bpc, repeat)
    if key in _NC_CACHE:
        return _NC_CACHE[key]
    global BPC
    old_bpc, BPC = BPC, bpc
    try:
        nc = bacc.Bacc("TRN2", target_bir_lowering=False, debug=False, num_devices=1)
        x = nc.dram_tensor("x", [bpc, C, N], f32, kind="ExternalInput").ap()
        gamma = nc.dram_tensor("gamma", [1], f32, kind="ExternalInput").ap()
        y = nc.dram_tensor("y", [bpc, C, N], f32, kind="ExternalOutput").ap()
        with tile.TileContext(nc) as tc:
            for _ in range(repeat):
                _build_body(nc, tc, x, gamma, y)
        nc.compile()
    finally:
        BPC = old_bpc
    _NC_CACHE[key] = nc
    return nc


def run(x, gamma, trace=False):
    """x: [B, C, H, W] f32, gamma: [1] f32 -> ([B, C, H, W] f32, results)"""
    x = np.ascontiguousarray(x, dtype=np.float32).reshape(B, C, N)
    gamma = np.ascontiguousarray(gamma, dtype=np.float32)
    nc = build_kernel()
    in_maps = [
        {"x": x[i * BPC:(i + 1) * BPC], "gamma": gamma} for i in range(NCORES)
    ]
    res = run_bass_kernel_spmd(nc, in_maps, core_ids=list(range(NCORES)),
                               trace=trace)
    out = np.concatenate([res.results[i]["y"] for i in range(NCORES)], axis=0)
    return out.reshape(B, C, H, W), res


def kernel(x, gamma):
    out, _ = run(x, gamma)
    return out
